# revision 21
# baseline (speedup 1.0000x reference)
"""nn_MultiHeadAttention_59253368815813 on 8 TRN2 NeuronCores.

The reference module is bug-faithful to its original nn.Module in two ways
that together collapse the computation:

  1. ``o = jnp.einsum('bhtl,bthd->bhtd', A, v)`` indexes ``v`` by the QUERY
     position ``t``, not the key position ``l``. ``l`` therefore only sums
     over the softmax weights, which sum to exactly 1 per row:
     ``o[b,h,t,d] == v[b,t,h,d]``. Q, K, the mask and the softmax never
     influence the output.
  2. ``o.reshape(b, T, d)`` with no transpose scrambles (head, token) so the
     reshaped activation row tj = 128*h + s is the concatenation over
     m=0..15 of v[b, 16*s+m, h, :].

So the exact computation is  out = scramble(x @ Wv) @ Wo.T,  and the
scramble makes output rows depend on one head only.

Sharding: 2 batches x 4 head-groups. Core c = (b=c//4, g=c%4) owns batch b
and heads {4g..4g+3} = Wv columns [256g, 256g+256) and output rows
[512g, 512g+512) of batch b. Each core loads only its batch's x (4.2MB in
bf16) instead of all of x, which is what made the previous version
DMA-bound (23.3MB/core at a shared ~360GB/s).

Per core, all in bf16 (PE runs bf16 at 1 cycle/row like f32r, but DMA
halves; quantization error ~2e-3 << the 2e-2 gate):
  stream x^T (tokens permuted to u = 128m + s, t = 16s + m) in 8 blocks of
  256 tokens; v-proj psum [128,256] per head-pair chases the stream; the
  reshape scramble happens in the psum->SBUF evacuation copies (spread over
  DVE/Pool/Act engines); output-projection columns [0,512) accumulate
  interleaved with the stream (chunk k uses only v tokens of block k);
  columns [512,1024) run as a second pass after the stream, overlapping the
  output DMAs.
"""

import sys
import types

import numpy as np

_TRN_REPO = "/opt/trn_rl_repo"
if _TRN_REPO not in sys.path:
    sys.path.insert(0, _TRN_REPO)


def _install_ntff_shim():
    """antenv.axon_hooks is absent in this container; provide it so
    BASS_TRACE=1 profiling works. No-op if the real module exists."""
    try:
        import antenv  # noqa: F401
    except ImportError:
        return
    if "antenv.axon_hooks" in sys.modules:
        return
    try:
        import antenv.axon_hooks  # noqa: F401
        return
    except ImportError:
        pass
    m = types.ModuleType("antenv.axon_hooks")
    m._hook = None
    m.set_axon_ntff_profile_hook = lambda h: setattr(m, "_hook", h)
    m.get_axon_ntff_profile_hook = lambda: m._hook
    sys.modules["antenv.axon_hooks"] = m
    try:
        from trn_agent_boot.trn_boot import _ntff_profile_via_ctypes

        hook = _ntff_profile_via_ctypes("/opt/axon/libaxon_pjrt.so")
        if hook is not None:
            m.set_axon_ntff_profile_hook(hook)
    except Exception:
        pass


_install_ntff_shim()

import ml_dtypes  # noqa: E402

import concourse.mybir as mybir  # noqa: E402
import concourse.tile as tile  # noqa: E402
from concourse import bacc  # noqa: E402
from concourse.bass_utils import run_bass_kernel_spmd  # noqa: E402

F32 = mybir.dt.float32
BF16 = mybir.dt.bfloat16
BF = ml_dtypes.bfloat16

B = 2
T = 2048
D = 1024
NCORES = 8
NB = 8       # 256-token (u) blocks per batch
UB = 256     # tokens per block
NC8 = 8      # contraction chunks (d = 8*128)
NH = 4       # local heads per core

_CACHED = None
LAST_RESULTS = None


def _build_module():
    nc = bacc.Bacc("TRN2", target_bir_lowering=False, debug=False,
                   num_devices=NCORES)

    xt_d = nc.dram_tensor("xt", [NB, 128, NC8, UB], BF16,
                          kind="ExternalInput").ap()
    wv_d = nc.dram_tensor("wv", [128, NC8, 256], BF16,
                          kind="ExternalInput").ap()
    wo_d = nc.dram_tensor("wo", [128, 8, D], BF16, kind="ExternalInput").ap()
    out_d = nc.dram_tensor("out", [NH, 128, D], BF16,
                           kind="ExternalOutput").ap()

    with tile.TileContext(nc) as tc:
        _emit(nc, tc, xt_d, wv_d, wo_d, out_d)
    nc.compile()
    return nc


def _emit(nc, tc, xt_d, wv_d, wo_d, out_d):
    from contextlib import ExitStack

    ctx = ExitStack()
    with ctx:
        wpool = ctx.enter_context(tc.tile_pool(name="w", bufs=1))
        xtp = ctx.enter_context(tc.tile_pool(name="xt", bufs=NB))
        vtp = ctx.enter_context(tc.tile_pool(name="vt", bufs=1))
        outp = ctx.enter_context(tc.tile_pool(name="outsb", bufs=4))
        ps_v = ctx.enter_context(tc.tile_pool(name="ps_v", bufs=4, space="PSUM"))
        ps_o = ctx.enter_context(tc.tile_pool(name="ps_o", bufs=4, space="PSUM"))

        # PE p-state warmup: the tensor engine clocks up only after ~3us of
        # continuous work, and the first real matmul cannot start until
        # ~10us (runtime preamble + first DMAs). Run throwaway matmuls on a
        # zeroed tile during that window so real matmuls run at full clock.
        warm_sb = wpool.tile([128, 256], BF16, tag="warm")
        nc.vector.memset(warm_sb[:], 0.0)
        warm_ps = ps_v.tile([128, UB], F32, tag="pv", name="warm_ps")
        for _ in range(8):
            nc.tensor.matmul(warm_ps[:], warm_sb[:, 0:128], warm_sb[:],
                             start=True, stop=True)

        wva = wpool.tile([128, 2, 256], BF16, tag="wva")
        wvb = wpool.tile([128, 2, 256], BF16, tag="wvb")
        wvc = wpool.tile([128, 4, 256], BF16, tag="wvc")

        def wv_lhs(c8, hp):
            t, i = (wva, c8) if c8 < 2 else (wvb, c8 - 2) if c8 < 4 \
                else (wvc, c8 - 4)
            return t[:, i, 128 * hp:128 * hp + 128]

        wo_sb = wpool.tile([128, 8, D], BF16, tag="wo")
        # block 0 is split in half-tiles so the first v-matmuls start after
        # only half of it (plus wva) has landed
        xt0a = xtp.tile([128, 4, UB], BF16, tag="xt0a")
        xt0b = xtp.tile([128, 4, UB], BF16, tag="xt0b")
        xts = [None] + [xtp.tile([128, NC8, UB], BF16, tag="xt",
                                 name=f"xt{k}") for k in range(1, NB)]

        def x_rhs(k, c8):
            if k == 0:
                t = xt0a if c8 < 4 else xt0b
                return t[:, c8 % 4, :]
            return xts[k][:, c8, :]

        # PE-gating transfers ride the sync queue in exact need-order (one
        # queue = deterministic priority); wo chunks go on the Act queue
        # since their deadlines are loose
        nc.sync.dma_start(wva[:], wv_d[:, 0:2, :])
        nc.sync.dma_start(xt0a[:], xt_d[0, :, 0:4, :])
        nc.sync.dma_start(wvb[:], wv_d[:, 2:4, :])
        nc.sync.dma_start(xt0b[:], xt_d[0, :, 4:8, :])
        nc.sync.dma_start(wvc[:], wv_d[:, 4:8, :])
        for k in range(1, NB):
            nc.sync.dma_start(xts[k][:], xt_d[k])
        for m2 in range(8):
            nc.scalar.dma_start(wo_sb[:, m2, :], wo_d[:, m2, :])

        # vt[h][64*(m%2)+di, 128*(m//2)+s] = v[t=16s+m, 256g+64h+di], bf16
        vt = [vtp.tile([128, D], BF16, tag=f"vt{h}", name=f"vt{h}")
              for h in range(NH)]

        psA = [ps_o.tile([128, 512], F32, tag="po", name=f"psA{h}")
               for h in range(NH)]

        # gpsimd cannot access PSUM, so evacuations go on DVE + Act only
        copy_engines = [nc.vector, nc.scalar]
        ce = [0]

        def ecopy(dst, src):
            eng = copy_engines[ce[0] % 2]
            ce[0] += 1
            if eng is nc.scalar:
                eng.copy(dst, src)
            else:
                eng.tensor_copy(dst, src)

        def vblock(k):
            psv = [ps_v.tile([128, UB], F32, tag="pv", name=f"pv{k}_{hp}")
                   for hp in range(2)]
            for c8 in range(NC8):
                for hp in range(2):
                    nc.tensor.matmul(psv[hp][:], wv_lhs(c8, hp),
                                     x_rhs(k, c8),
                                     start=(c8 == 0), stop=(c8 == NC8 - 1))
            return psv

        def evac(k, psv):
            # block k holds m in {2k, 2k+1}; j = m%2 = local u//128
            for hp in range(2):
                for hh in range(2):
                    for j in range(2):
                        ecopy(vt[2 * hp + hh][64 * j:64 * j + 64,
                                              128 * k:128 * k + 128],
                              psv[hp][64 * hh:64 * hh + 64,
                                      128 * j:128 * j + 128])

        def outA(k):
            for h in range(NH):
                nc.tensor.matmul(psA[h][:], vt[h][:, 128 * k:128 * k + 128],
                                 wo_sb[:, k, 0:512],
                                 start=(k == 0), stop=(k == NB - 1))

        def flushA(h):
            ob = outp.tile([128, 512], BF16, tag="ob", name=f"obA{h}")
            ecopy(ob[:], psA[h][:])
            nc.scalar.dma_start(out_d[h, :, 0:512], ob[:])

        # stream: v-proj chases x DMAs; out-proj chunk k-1 fills PE slack
        psv_prev = vblock(0)
        evac(0, psv_prev)
        for k in range(1, NB):
            psv = vblock(k)
            evac(k, psv)
            outA(k - 1)
        outA(NB - 1)

        # queue all psA evacuations first so the psB bank-reuse waits clear
        # while the first psB groups are still accumulating
        for h in range(NH):
            flushA(h)

        # second pass: out-proj columns [512,1024) + drains
        for h in range(NH):
            psB = ps_o.tile([128, 512], F32, tag="po", name=f"psB{h}")
            for m2 in range(8):
                nc.tensor.matmul(psB[:], vt[h][:, 128 * m2:128 * m2 + 128],
                                 wo_sb[:, m2, 512:1024],
                                 start=(m2 == 0), stop=(m2 == 7))
            ob = outp.tile([128, 512], BF16, tag="ob", name=f"obB{h}")
            ecopy(ob[:], psB[:])
            nc.scalar.dma_start(out_d[h, :, 512:1024], ob[:])


def _get_module():
    global _CACHED
    if _CACHED is None:
        _CACHED = _build_module()
    return _CACHED


def kernel(x, mask, Wq, Wk, Wv, Wo):
    global LAST_RESULTS
    x = np.asarray(x, dtype=np.float32)
    Wv = np.asarray(Wv, dtype=np.float32)
    Wo = np.asarray(Wo, dtype=np.float32)

    b, t, d = x.shape
    assert (b, t, d) == (B, T, D), (b, t, d)

    # x^T with tokens permuted to u = 128m + s (original t = 16s + m),
    # laid out [k, p, c8, u] to match the SBUF tiles exactly
    xts = []
    for bb in range(B):
        xT = x[bb].T                                      # [d, t]
        xTp = xT.reshape(D, 128, 16).transpose(0, 2, 1).reshape(D, T)
        xt = xTp.reshape(NC8, 128, NB, UB).transpose(2, 1, 0, 3)
        xts.append(np.ascontiguousarray(xt).astype(BF))

    # wv[p, c8, col] = Wv[128*c8 + p, col]; per-core slice of 256 cols
    wvp = Wv.reshape(NC8, 128, D).transpose(1, 0, 2)
    # wo[p, m2, n] = Wo.T[128*m2 + p, n]
    woT = np.ascontiguousarray(
        Wo.T.reshape(8, 128, D).transpose(1, 0, 2)).astype(BF)

    in_maps = []
    for c in range(NCORES):
        bb, g = c // 4, c % 4
        in_maps.append({
            "xt": xts[bb],
            "wv": np.ascontiguousarray(
                wvp[:, :, 256 * g:256 * g + 256]).astype(BF),
            "wo": woT,
        })

    nc = _get_module()
    res = run_bass_kernel_spmd(nc, in_maps, list(range(NCORES)))
    LAST_RESULTS = res

    out = np.empty((B, T, D), np.float32)
    for c in range(NCORES):
        bb, g = c // 4, c % 4
        out[bb, 512 * g:512 * g + 512, :] = \
            np.asarray(res.results[c]["out"]).astype(np.float32).reshape(512, D)
    return out


# revision 22
# speedup vs baseline: 1.0583x; 1.0583x over previous
"""nn_MultiHeadAttention_59253368815813 on 8 TRN2 NeuronCores.

The reference module is bug-faithful to its original nn.Module in two ways
that together collapse the computation:

  1. ``o = jnp.einsum('bhtl,bthd->bhtd', A, v)`` indexes ``v`` by the QUERY
     position ``t``, not the key position ``l``. ``l`` therefore only sums
     over the softmax weights, which sum to exactly 1 per row:
     ``o[b,h,t,d] == v[b,t,h,d]``. Q, K, the mask and the softmax never
     influence the output.
  2. ``o.reshape(b, T, d)`` with no transpose scrambles (head, token) so the
     reshaped activation row tj = 128*h + s is the concatenation over
     m=0..15 of v[b, 16*s+m, h, :].

So the exact computation is  out = scramble(x @ Wv) @ Wo.T,  and the
scramble makes output rows depend on one head only.

Sharding: 2 batches x 4 head-groups. Core c = (b=c//4, g=c%4) owns batch b
and heads {4g..4g+3} = Wv columns [256g, 256g+256) and output rows
[512g, 512g+512) of batch b. Each core loads only its batch's x (4.2MB in
bf16) instead of all of x, which is what made the previous version
DMA-bound (23.3MB/core at a shared ~360GB/s).

Per core, all in bf16 (PE runs bf16 at 1 cycle/row like f32r, but DMA
halves; quantization error ~2e-3 << the 2e-2 gate):
  stream x^T (tokens permuted to u = 128m + s, t = 16s + m) in 8 blocks of
  256 tokens; v-proj psum [128,256] per head-pair chases the stream; the
  reshape scramble happens in the psum->SBUF evacuation copies (spread over
  DVE/Pool/Act engines); output-projection columns [0,512) accumulate
  interleaved with the stream (chunk k uses only v tokens of block k);
  columns [512,1024) run as a second pass after the stream, overlapping the
  output DMAs.
"""

import sys
import types

import numpy as np

_TRN_REPO = "/opt/trn_rl_repo"
if _TRN_REPO not in sys.path:
    sys.path.insert(0, _TRN_REPO)


def _install_ntff_shim():
    """antenv.axon_hooks is absent in this container; provide it so
    BASS_TRACE=1 profiling works. No-op if the real module exists."""
    try:
        import antenv  # noqa: F401
    except ImportError:
        return
    if "antenv.axon_hooks" in sys.modules:
        return
    try:
        import antenv.axon_hooks  # noqa: F401
        return
    except ImportError:
        pass
    m = types.ModuleType("antenv.axon_hooks")
    m._hook = None
    m.set_axon_ntff_profile_hook = lambda h: setattr(m, "_hook", h)
    m.get_axon_ntff_profile_hook = lambda: m._hook
    sys.modules["antenv.axon_hooks"] = m
    try:
        from trn_agent_boot.trn_boot import _ntff_profile_via_ctypes

        hook = _ntff_profile_via_ctypes("/opt/axon/libaxon_pjrt.so")
        if hook is not None:
            m.set_axon_ntff_profile_hook(hook)
    except Exception:
        pass


_install_ntff_shim()

import ml_dtypes  # noqa: E402

import concourse.mybir as mybir  # noqa: E402
import concourse.tile as tile  # noqa: E402
from concourse import bacc  # noqa: E402
from concourse.bass_utils import run_bass_kernel_spmd  # noqa: E402

F32 = mybir.dt.float32
BF16 = mybir.dt.bfloat16
BF = ml_dtypes.bfloat16

B = 2
T = 2048
D = 1024
NCORES = 8
NB = 8       # 256-token (u) blocks per batch
UB = 256     # tokens per block
NC8 = 8      # contraction chunks (d = 8*128)
NH = 4       # local heads per core

_CACHED = None
LAST_RESULTS = None


def _build_module():
    nc = bacc.Bacc("TRN2", target_bir_lowering=False, debug=False,
                   num_devices=NCORES)

    xt_d = nc.dram_tensor("xt", [NB, 128, NC8, UB], BF16,
                          kind="ExternalInput").ap()
    wv_d = nc.dram_tensor("wv", [128, NC8, 256], BF16,
                          kind="ExternalInput").ap()
    wo_d = nc.dram_tensor("wo", [128, 8, D], BF16, kind="ExternalInput").ap()
    out_d = nc.dram_tensor("out", [NH, 128, D], BF16,
                           kind="ExternalOutput").ap()

    with tile.TileContext(nc) as tc:
        _emit(nc, tc, xt_d, wv_d, wo_d, out_d)
    nc.compile()
    return nc


def _emit(nc, tc, xt_d, wv_d, wo_d, out_d):
    from contextlib import ExitStack

    ctx = ExitStack()
    with ctx:
        wpool = ctx.enter_context(tc.tile_pool(name="w", bufs=1))
        xtp = ctx.enter_context(tc.tile_pool(name="xt", bufs=NB))
        vtp = ctx.enter_context(tc.tile_pool(name="vt", bufs=1))
        outp = ctx.enter_context(tc.tile_pool(name="outsb", bufs=4))
        ps_v = ctx.enter_context(tc.tile_pool(name="ps_v", bufs=4, space="PSUM"))
        ps_o = ctx.enter_context(tc.tile_pool(name="ps_o", bufs=4, space="PSUM"))

        # PE p-state warmup: the tensor engine clocks up only after ~3us of
        # continuous work, and the first real matmul cannot start until
        # ~10us (runtime preamble + first DMAs). Run throwaway matmuls on a
        # zeroed tile during that window so real matmuls run at full clock.
        warm_sb = wpool.tile([128, 256], BF16, tag="warm")
        nc.vector.memset(warm_sb[:], 0.0)
        warm_ps = ps_v.tile([128, UB], F32, tag="pv", name="warm_ps")
        for _ in range(8):
            nc.tensor.matmul(warm_ps[:], warm_sb[:, 0:128], warm_sb[:],
                             start=True, stop=True)

        wva = wpool.tile([128, 2, 256], BF16, tag="wva")
        wvb = wpool.tile([128, 2, 256], BF16, tag="wvb")
        wvc = wpool.tile([128, 4, 256], BF16, tag="wvc")

        def wv_lhs(c8, hp):
            t, i = (wva, c8) if c8 < 2 else (wvb, c8 - 2) if c8 < 4 \
                else (wvc, c8 - 4)
            return t[:, i, 128 * hp:128 * hp + 128]

        wo_sb = wpool.tile([128, 8, D], BF16, tag="wo")
        # block 0 is split in half-tiles so the first v-matmuls start after
        # only half of it (plus wva) has landed
        xt0a = xtp.tile([128, 4, UB], BF16, tag="xt0a")
        xt0b = xtp.tile([128, 4, UB], BF16, tag="xt0b")
        xts = [None] + [xtp.tile([128, NC8, UB], BF16, tag="xt",
                                 name=f"xt{k}") for k in range(1, NB)]

        def x_rhs(k, c8):
            if k == 0:
                t = xt0a if c8 < 4 else xt0b
                return t[:, c8 % 4, :]
            return xts[k][:, c8, :]

        # PE-gating transfers ride the sync queue in exact need-order (one
        # queue = deterministic priority); wo chunks go on the Act queue
        # since their deadlines are loose
        nc.sync.dma_start(wva[:], wv_d[:, 0:2, :])
        nc.sync.dma_start(xt0a[:], xt_d[0, :, 0:4, :])
        nc.sync.dma_start(wvb[:], wv_d[:, 2:4, :])
        nc.sync.dma_start(xt0b[:], xt_d[0, :, 4:8, :])
        nc.sync.dma_start(wvc[:], wv_d[:, 4:8, :])
        nc.sync.dma_start(xts[1][:], xt_d[1])
        nc.sync.dma_start(xts[2][:], xt_d[2])
        # x block k+3 and wo chunk k: x keeps a two-slot lead over wo
        for k in range(NB - 3):
            nc.sync.dma_start(xts[k + 3][:], xt_d[k + 3])
            nc.sync.dma_start(wo_sb[:, k, :], wo_d[:, k, :])
        for m2 in range(NB - 3, 8):
            nc.sync.dma_start(wo_sb[:, m2, :], wo_d[:, m2, :])

        # vt[h][64*(m%2)+di, 128*(m//2)+s] = v[t=16s+m, 256g+64h+di], bf16
        vt = [vtp.tile([128, D], BF16, tag=f"vt{h}", name=f"vt{h}")
              for h in range(NH)]

        psA = [ps_o.tile([128, 512], F32, tag="po", name=f"psA{h}")
               for h in range(NH)]

        # gpsimd cannot access PSUM, so evacuations go on DVE + Act only
        copy_engines = [nc.vector, nc.scalar]
        ce = [0]

        def ecopy(dst, src):
            eng = copy_engines[ce[0] % 2]
            ce[0] += 1
            if eng is nc.scalar:
                eng.copy(dst, src)
            else:
                eng.tensor_copy(dst, src)

        def vblock(k):
            psv = [ps_v.tile([128, UB], F32, tag="pv", name=f"pv{k}_{hp}")
                   for hp in range(2)]
            for c8 in range(NC8):
                for hp in range(2):
                    nc.tensor.matmul(psv[hp][:], wv_lhs(c8, hp),
                                     x_rhs(k, c8),
                                     start=(c8 == 0), stop=(c8 == NC8 - 1))
            return psv

        def evac(k, psv):
            # block k holds m in {2k, 2k+1}; j = m%2 = local u//128
            for hp in range(2):
                for hh in range(2):
                    for j in range(2):
                        ecopy(vt[2 * hp + hh][64 * j:64 * j + 64,
                                              128 * k:128 * k + 128],
                              psv[hp][64 * hh:64 * hh + 64,
                                      128 * j:128 * j + 128])

        def outA(k):
            for h in range(NH):
                nc.tensor.matmul(psA[h][:], vt[h][:, 128 * k:128 * k + 128],
                                 wo_sb[:, k, 0:512],
                                 start=(k == 0), stop=(k == NB - 1))

        def flushA(h):
            ob = outp.tile([128, 512], BF16, tag="ob", name=f"obA{h}")
            ecopy(ob[:], psA[h][:])
            nc.scalar.dma_start(out_d[h, :, 0:512], ob[:])

        # stream: v-proj chases x DMAs; out-proj chunk k-1 fills PE slack
        psv_prev = vblock(0)
        evac(0, psv_prev)
        for k in range(1, NB):
            psv = vblock(k)
            evac(k, psv)
            outA(k - 1)
        outA(NB - 1)

        # queue all psA evacuations first so the psB bank-reuse waits clear
        # while the first psB groups are still accumulating
        for h in range(NH):
            flushA(h)

        # second pass: out-proj columns [512,1024) + drains
        for h in range(NH):
            psB = ps_o.tile([128, 512], F32, tag="po", name=f"psB{h}")
            for m2 in range(8):
                nc.tensor.matmul(psB[:], vt[h][:, 128 * m2:128 * m2 + 128],
                                 wo_sb[:, m2, 512:1024],
                                 start=(m2 == 0), stop=(m2 == 7))
            ob = outp.tile([128, 512], BF16, tag="ob", name=f"obB{h}")
            ecopy(ob[:], psB[:])
            nc.scalar.dma_start(out_d[h, :, 512:1024], ob[:])


def _get_module():
    global _CACHED
    if _CACHED is None:
        _CACHED = _build_module()
    return _CACHED


def kernel(x, mask, Wq, Wk, Wv, Wo):
    global LAST_RESULTS
    x = np.asarray(x, dtype=np.float32)
    Wv = np.asarray(Wv, dtype=np.float32)
    Wo = np.asarray(Wo, dtype=np.float32)

    b, t, d = x.shape
    assert (b, t, d) == (B, T, D), (b, t, d)

    # x^T with tokens permuted to u = 128m + s (original t = 16s + m),
    # laid out [k, p, c8, u] to match the SBUF tiles exactly
    xts = []
    for bb in range(B):
        xT = x[bb].T                                      # [d, t]
        xTp = xT.reshape(D, 128, 16).transpose(0, 2, 1).reshape(D, T)
        xt = xTp.reshape(NC8, 128, NB, UB).transpose(2, 1, 0, 3)
        xts.append(np.ascontiguousarray(xt).astype(BF))

    # wv[p, c8, col] = Wv[128*c8 + p, col]; per-core slice of 256 cols
    wvp = Wv.reshape(NC8, 128, D).transpose(1, 0, 2)
    # wo[p, m2, n] = Wo.T[128*m2 + p, n]
    woT = np.ascontiguousarray(
        Wo.T.reshape(8, 128, D).transpose(1, 0, 2)).astype(BF)

    in_maps = []
    for c in range(NCORES):
        bb, g = c // 4, c % 4
        in_maps.append({
            "xt": xts[bb],
            "wv": np.ascontiguousarray(
                wvp[:, :, 256 * g:256 * g + 256]).astype(BF),
            "wo": woT,
        })

    nc = _get_module()
    res = run_bass_kernel_spmd(nc, in_maps, list(range(NCORES)))
    LAST_RESULTS = res

    out = np.empty((B, T, D), np.float32)
    for c in range(NCORES):
        bb, g = c // 4, c % 4
        out[bb, 512 * g:512 * g + 512, :] = \
            np.asarray(res.results[c]["out"]).astype(np.float32).reshape(512, D)
    return out


# revision 23
# speedup vs baseline: 1.0822x; 1.0225x over previous
"""nn_MultiHeadAttention_59253368815813 on 8 TRN2 NeuronCores.

The reference module is bug-faithful to its original nn.Module in two ways
that together collapse the computation:

  1. ``o = jnp.einsum('bhtl,bthd->bhtd', A, v)`` indexes ``v`` by the QUERY
     position ``t``, not the key position ``l``. ``l`` therefore only sums
     over the softmax weights, which sum to exactly 1 per row:
     ``o[b,h,t,d] == v[b,t,h,d]``. Q, K, the mask and the softmax never
     influence the output.
  2. ``o.reshape(b, T, d)`` with no transpose scrambles (head, token) so the
     reshaped activation row tj = 128*h + s is the concatenation over
     m=0..15 of v[b, 16*s+m, h, :].

So the exact computation is  out = scramble(x @ Wv) @ Wo.T,  and the
scramble makes output rows depend on one head only.

Sharding: 2 batches x 4 head-groups. Core c = (b=c//4, g=c%4) owns batch b
and heads {4g..4g+3} = Wv columns [256g, 256g+256) and output rows
[512g, 512g+512) of batch b. Each core loads only its batch's x (4.2MB in
bf16) instead of all of x, which is what made the previous version
DMA-bound (23.3MB/core at a shared ~360GB/s).

Per core, all in bf16 (PE runs bf16 at 1 cycle/row like f32r, but DMA
halves; quantization error ~2e-3 << the 2e-2 gate):
  stream x^T (tokens permuted to u = 128m + s, t = 16s + m) in 8 blocks of
  256 tokens; v-proj psum [128,256] per head-pair chases the stream; the
  reshape scramble happens in the psum->SBUF evacuation copies (spread over
  DVE/Pool/Act engines); output-projection columns [0,512) accumulate
  interleaved with the stream (chunk k uses only v tokens of block k);
  columns [512,1024) run as a second pass after the stream, overlapping the
  output DMAs.
"""

import sys
import types

import numpy as np

_TRN_REPO = "/opt/trn_rl_repo"
if _TRN_REPO not in sys.path:
    sys.path.insert(0, _TRN_REPO)


def _install_ntff_shim():
    """antenv.axon_hooks is absent in this container; provide it so
    BASS_TRACE=1 profiling works. No-op if the real module exists."""
    try:
        import antenv  # noqa: F401
    except ImportError:
        return
    if "antenv.axon_hooks" in sys.modules:
        return
    try:
        import antenv.axon_hooks  # noqa: F401
        return
    except ImportError:
        pass
    m = types.ModuleType("antenv.axon_hooks")
    m._hook = None
    m.set_axon_ntff_profile_hook = lambda h: setattr(m, "_hook", h)
    m.get_axon_ntff_profile_hook = lambda: m._hook
    sys.modules["antenv.axon_hooks"] = m
    try:
        from trn_agent_boot.trn_boot import _ntff_profile_via_ctypes

        hook = _ntff_profile_via_ctypes("/opt/axon/libaxon_pjrt.so")
        if hook is not None:
            m.set_axon_ntff_profile_hook(hook)
    except Exception:
        pass


_install_ntff_shim()

import ml_dtypes  # noqa: E402

import concourse.mybir as mybir  # noqa: E402
import concourse.tile as tile  # noqa: E402
from concourse import bacc  # noqa: E402
from concourse.bass_utils import run_bass_kernel_spmd  # noqa: E402

F32 = mybir.dt.float32
BF16 = mybir.dt.bfloat16
BF = ml_dtypes.bfloat16

B = 2
T = 2048
D = 1024
NCORES = 8
NB = 8       # 256-token (u) blocks per batch
UB = 256     # tokens per block
NC8 = 8      # contraction chunks (d = 8*128)
NH = 4       # local heads per core

_CACHED = None
LAST_RESULTS = None


def _build_module():
    nc = bacc.Bacc("TRN2", target_bir_lowering=False, debug=False,
                   num_devices=NCORES)

    xt_d = nc.dram_tensor("xt", [NB, 128, NC8, UB], BF16,
                          kind="ExternalInput").ap()
    wv_d = nc.dram_tensor("wv", [128, NC8, 256], BF16,
                          kind="ExternalInput").ap()
    wo_d = nc.dram_tensor("wo", [128, 8, D], BF16, kind="ExternalInput").ap()
    out_d = nc.dram_tensor("out", [NH, 128, D], BF16,
                           kind="ExternalOutput").ap()

    with tile.TileContext(nc) as tc:
        _emit(nc, tc, xt_d, wv_d, wo_d, out_d)
    nc.compile()
    return nc


def _emit(nc, tc, xt_d, wv_d, wo_d, out_d):
    from contextlib import ExitStack

    ctx = ExitStack()
    with ctx:
        wpool = ctx.enter_context(tc.tile_pool(name="w", bufs=1))
        xtp = ctx.enter_context(tc.tile_pool(name="xt", bufs=NB))
        vtp = ctx.enter_context(tc.tile_pool(name="vt", bufs=1))
        outp = ctx.enter_context(tc.tile_pool(name="outsb", bufs=4))
        ps_v = ctx.enter_context(tc.tile_pool(name="ps_v", bufs=4, space="PSUM"))
        ps_o = ctx.enter_context(tc.tile_pool(name="ps_o", bufs=4, space="PSUM"))

        # PE p-state warmup: the tensor engine clocks up only after ~3us of
        # continuous work, and the first real matmul cannot start until
        # ~10us (runtime preamble + first DMAs). Run throwaway matmuls on a
        # zeroed tile during that window so real matmuls run at full clock.
        warm_sb = wpool.tile([128, 256], BF16, tag="warm")
        nc.vector.memset(warm_sb[:], 0.0)
        warm_ps = ps_v.tile([128, UB], F32, tag="pv", name="warm_ps")
        for _ in range(16):
            nc.tensor.matmul(warm_ps[:], warm_sb[:, 0:128], warm_sb[:],
                             start=True, stop=True)

        wva = wpool.tile([128, 2, 256], BF16, tag="wva")
        wvb = wpool.tile([128, 2, 256], BF16, tag="wvb")
        wvc = wpool.tile([128, 4, 256], BF16, tag="wvc")

        def wv_lhs(c8, hp):
            t, i = (wva, c8) if c8 < 2 else (wvb, c8 - 2) if c8 < 4 \
                else (wvc, c8 - 4)
            return t[:, i, 128 * hp:128 * hp + 128]

        wo_sb = wpool.tile([128, 8, D], BF16, tag="wo")
        # block 0 is split in half-tiles so the first v-matmuls start after
        # only half of it (plus wva) has landed
        xt0a = xtp.tile([128, 4, UB], BF16, tag="xt0a")
        xt0b = xtp.tile([128, 4, UB], BF16, tag="xt0b")
        xts = [None] + [xtp.tile([128, NC8, UB], BF16, tag="xt",
                                 name=f"xt{k}") for k in range(1, NB)]

        def x_rhs(k, c8):
            if k == 0:
                t = xt0a if c8 < 4 else xt0b
                return t[:, c8 % 4, :]
            return xts[k][:, c8, :]

        # PE-gating transfers ride the sync queue in exact need-order (one
        # queue = deterministic priority); wo chunks go on the Act queue
        # since their deadlines are loose
        nc.sync.dma_start(wva[:], wv_d[:, 0:2, :])
        nc.sync.dma_start(xt0a[:], xt_d[0, :, 0:4, :])
        nc.sync.dma_start(wvb[:], wv_d[:, 2:4, :])
        nc.sync.dma_start(xt0b[:], xt_d[0, :, 4:8, :])
        nc.sync.dma_start(wvc[:], wv_d[:, 4:8, :])
        nc.sync.dma_start(xts[1][:], xt_d[1])
        nc.sync.dma_start(xts[2][:], xt_d[2])
        # x block k+3 and wo chunk k: x keeps a two-slot lead over wo
        for k in range(NB - 3):
            nc.sync.dma_start(xts[k + 3][:], xt_d[k + 3])
            nc.sync.dma_start(wo_sb[:, k, :], wo_d[:, k, :])
        for m2 in range(NB - 3, 8):
            nc.sync.dma_start(wo_sb[:, m2, :], wo_d[:, m2, :])

        # vt[h][64*(m%2)+di, 128*(m//2)+s] = v[t=16s+m, 256g+64h+di], bf16
        vt = [vtp.tile([128, D], BF16, tag=f"vt{h}", name=f"vt{h}")
              for h in range(NH)]

        psA = [ps_o.tile([128, 512], F32, tag="po", name=f"psA{h}")
               for h in range(NH)]

        # gpsimd cannot access PSUM, so evacuations go on DVE + Act only
        copy_engines = [nc.vector, nc.scalar]
        ce = [0]

        def ecopy(dst, src):
            eng = copy_engines[ce[0] % 2]
            ce[0] += 1
            if eng is nc.scalar:
                eng.copy(dst, src)
            else:
                eng.tensor_copy(dst, src)

        def vblock(k):
            psv = [ps_v.tile([128, UB], F32, tag="pv", name=f"pv{k}_{hp}")
                   for hp in range(2)]
            for c8 in range(NC8):
                for hp in range(2):
                    nc.tensor.matmul(psv[hp][:], wv_lhs(c8, hp),
                                     x_rhs(k, c8),
                                     start=(c8 == 0), stop=(c8 == NC8 - 1))
            return psv

        def evac(k, psv):
            # block k holds m in {2k, 2k+1}; j = m%2 = local u//128
            for hp in range(2):
                for hh in range(2):
                    for j in range(2):
                        ecopy(vt[2 * hp + hh][64 * j:64 * j + 64,
                                              128 * k:128 * k + 128],
                              psv[hp][64 * hh:64 * hh + 64,
                                      128 * j:128 * j + 128])

        def outA(k):
            for h in range(NH):
                nc.tensor.matmul(psA[h][:], vt[h][:, 128 * k:128 * k + 128],
                                 wo_sb[:, k, 0:512],
                                 start=(k == 0), stop=(k == NB - 1))

        def flushA(h):
            ob = outp.tile([128, 512], BF16, tag="ob", name=f"obA{h}")
            ecopy(ob[:], psA[h][:])
            nc.scalar.dma_start(out_d[h, :, 0:512], ob[:])

        # stream: v-proj chases x DMAs; out-proj chunk k-1 fills PE slack
        psv_prev = vblock(0)
        evac(0, psv_prev)
        for k in range(1, NB):
            psv = vblock(k)
            evac(k, psv)
            outA(k - 1)
        outA(NB - 1)

        # queue all psA evacuations first so the psB bank-reuse waits clear
        # while the first psB groups are still accumulating
        for h in range(NH):
            flushA(h)

        # second pass: out-proj columns [512,1024) + drains
        for h in range(NH):
            psB = ps_o.tile([128, 512], F32, tag="po", name=f"psB{h}")
            for m2 in range(8):
                nc.tensor.matmul(psB[:], vt[h][:, 128 * m2:128 * m2 + 128],
                                 wo_sb[:, m2, 512:1024],
                                 start=(m2 == 0), stop=(m2 == 7))
            ob = outp.tile([128, 512], BF16, tag="ob", name=f"obB{h}")
            ecopy(ob[:], psB[:])
            nc.scalar.dma_start(out_d[h, :, 512:1024], ob[:])


def _get_module():
    global _CACHED
    if _CACHED is None:
        _CACHED = _build_module()
    return _CACHED


def kernel(x, mask, Wq, Wk, Wv, Wo):
    global LAST_RESULTS
    x = np.asarray(x, dtype=np.float32)
    Wv = np.asarray(Wv, dtype=np.float32)
    Wo = np.asarray(Wo, dtype=np.float32)

    b, t, d = x.shape
    assert (b, t, d) == (B, T, D), (b, t, d)

    # x^T with tokens permuted to u = 128m + s (original t = 16s + m),
    # laid out [k, p, c8, u] to match the SBUF tiles exactly
    xts = []
    for bb in range(B):
        xT = x[bb].T                                      # [d, t]
        xTp = xT.reshape(D, 128, 16).transpose(0, 2, 1).reshape(D, T)
        xt = xTp.reshape(NC8, 128, NB, UB).transpose(2, 1, 0, 3)
        xts.append(np.ascontiguousarray(xt).astype(BF))

    # wv[p, c8, col] = Wv[128*c8 + p, col]; per-core slice of 256 cols
    wvp = Wv.reshape(NC8, 128, D).transpose(1, 0, 2)
    # wo[p, m2, n] = Wo.T[128*m2 + p, n]
    woT = np.ascontiguousarray(
        Wo.T.reshape(8, 128, D).transpose(1, 0, 2)).astype(BF)

    in_maps = []
    for c in range(NCORES):
        bb, g = c // 4, c % 4
        in_maps.append({
            "xt": xts[bb],
            "wv": np.ascontiguousarray(
                wvp[:, :, 256 * g:256 * g + 256]).astype(BF),
            "wo": woT,
        })

    nc = _get_module()
    res = run_bass_kernel_spmd(nc, in_maps, list(range(NCORES)))
    LAST_RESULTS = res

    out = np.empty((B, T, D), np.float32)
    for c in range(NCORES):
        bb, g = c // 4, c % 4
        out[bb, 512 * g:512 * g + 512, :] = \
            np.asarray(res.results[c]["out"]).astype(np.float32).reshape(512, D)
    return out


# revision 24
# speedup vs baseline: 1.0937x; 1.0107x over previous
"""nn_MultiHeadAttention_59253368815813 on 8 TRN2 NeuronCores.

The reference module is bug-faithful to its original nn.Module in two ways
that together collapse the computation:

  1. ``o = jnp.einsum('bhtl,bthd->bhtd', A, v)`` indexes ``v`` by the QUERY
     position ``t``, not the key position ``l``. ``l`` therefore only sums
     over the softmax weights, which sum to exactly 1 per row:
     ``o[b,h,t,d] == v[b,t,h,d]``. Q, K, the mask and the softmax never
     influence the output.
  2. ``o.reshape(b, T, d)`` with no transpose scrambles (head, token) so the
     reshaped activation row tj = 128*h + s is the concatenation over
     m=0..15 of v[b, 16*s+m, h, :].

So the exact computation is  out = scramble(x @ Wv) @ Wo.T,  and the
scramble makes output rows depend on one head only.

Sharding: 2 batches x 4 head-groups. Core c = (b=c//4, g=c%4) owns batch b
and heads {4g..4g+3} = Wv columns [256g, 256g+256) and output rows
[512g, 512g+512) of batch b. Each core loads only its batch's x (4.2MB in
bf16) instead of all of x, which is what made the previous version
DMA-bound (23.3MB/core at a shared ~360GB/s).

Per core, all in bf16 (PE runs bf16 at 1 cycle/row like f32r, but DMA
halves; quantization error ~2e-3 << the 2e-2 gate):
  stream x^T (tokens permuted to u = 128m + s, t = 16s + m) in 8 blocks of
  256 tokens; v-proj psum [128,256] per head-pair chases the stream; the
  reshape scramble happens in the psum->SBUF evacuation copies (spread over
  DVE/Pool/Act engines); output-projection columns [0,512) accumulate
  interleaved with the stream (chunk k uses only v tokens of block k);
  columns [512,1024) run as a second pass after the stream, overlapping the
  output DMAs.
"""

import sys
import types

import numpy as np

_TRN_REPO = "/opt/trn_rl_repo"
if _TRN_REPO not in sys.path:
    sys.path.insert(0, _TRN_REPO)


def _install_ntff_shim():
    """antenv.axon_hooks is absent in this container; provide it so
    BASS_TRACE=1 profiling works. No-op if the real module exists."""
    try:
        import antenv  # noqa: F401
    except ImportError:
        return
    if "antenv.axon_hooks" in sys.modules:
        return
    try:
        import antenv.axon_hooks  # noqa: F401
        return
    except ImportError:
        pass
    m = types.ModuleType("antenv.axon_hooks")
    m._hook = None
    m.set_axon_ntff_profile_hook = lambda h: setattr(m, "_hook", h)
    m.get_axon_ntff_profile_hook = lambda: m._hook
    sys.modules["antenv.axon_hooks"] = m
    try:
        from trn_agent_boot.trn_boot import _ntff_profile_via_ctypes

        hook = _ntff_profile_via_ctypes("/opt/axon/libaxon_pjrt.so")
        if hook is not None:
            m.set_axon_ntff_profile_hook(hook)
    except Exception:
        pass


_install_ntff_shim()

import ml_dtypes  # noqa: E402

import concourse.mybir as mybir  # noqa: E402
import concourse.tile as tile  # noqa: E402
from concourse import bacc  # noqa: E402
from concourse.bass_utils import run_bass_kernel_spmd  # noqa: E402

F32 = mybir.dt.float32
BF16 = mybir.dt.bfloat16
BF = ml_dtypes.bfloat16

B = 2
T = 2048
D = 1024
NCORES = 8
NB = 8       # 256-token (u) blocks per batch
UB = 256     # tokens per block
NC8 = 8      # contraction chunks (d = 8*128)
NH = 4       # local heads per core

_CACHED = None
LAST_RESULTS = None


def _build_module():
    nc = bacc.Bacc("TRN2", target_bir_lowering=False, debug=False,
                   num_devices=NCORES)

    xt_d = nc.dram_tensor("xt", [NB, 128, NC8, UB], BF16,
                          kind="ExternalInput").ap()
    wv_d = nc.dram_tensor("wv", [128, NC8, 256], BF16,
                          kind="ExternalInput").ap()
    wo_d = nc.dram_tensor("wo", [128, 8, D], BF16, kind="ExternalInput").ap()
    out_d = nc.dram_tensor("out", [NH, 128, D], BF16,
                           kind="ExternalOutput").ap()

    with tile.TileContext(nc) as tc:
        _emit(nc, tc, xt_d, wv_d, wo_d, out_d)
    nc.compile()
    return nc


def _emit(nc, tc, xt_d, wv_d, wo_d, out_d):
    from contextlib import ExitStack

    ctx = ExitStack()
    with ctx:
        wpool = ctx.enter_context(tc.tile_pool(name="w", bufs=1))
        xtp = ctx.enter_context(tc.tile_pool(name="xt", bufs=NB))
        vtp = ctx.enter_context(tc.tile_pool(name="vt", bufs=1))
        outp = ctx.enter_context(tc.tile_pool(name="outsb", bufs=4))
        ps_v = ctx.enter_context(tc.tile_pool(name="ps_v", bufs=4, space="PSUM"))
        ps_o = ctx.enter_context(tc.tile_pool(name="ps_o", bufs=4, space="PSUM"))

        # PE p-state warmup: the tensor engine clocks up only after ~3us of
        # continuous work, and the first real matmul cannot start until
        # ~10us (runtime preamble + first DMAs). Run throwaway matmuls on a
        # zeroed tile during that window so real matmuls run at full clock.
        warm_sb = wpool.tile([128, 256], BF16, tag="warm")
        nc.vector.memset(warm_sb[:], 0.0)
        warm_ps = ps_v.tile([128, 512], F32, tag="pv", name="warm_ps")
        for _ in range(16):
            nc.tensor.matmul(warm_ps[:, 0:256], warm_sb[:, 0:128],
                             warm_sb[:], start=True, stop=True)

        wva = wpool.tile([128, 2, 256], BF16, tag="wva")
        wvb = wpool.tile([128, 2, 256], BF16, tag="wvb")
        wvc = wpool.tile([128, 4, 256], BF16, tag="wvc")

        def wv_lhs(c8, hp):
            t, i = (wva, c8) if c8 < 2 else (wvb, c8 - 2) if c8 < 4 \
                else (wvc, c8 - 4)
            return t[:, i, 128 * hp:128 * hp + 128]

        wo_sb = wpool.tile([128, 8, D], BF16, tag="wo")
        # block 0 is split in half-tiles so the first v-matmuls start after
        # only half of it (plus wva) has landed
        xt0a = xtp.tile([128, 4, UB], BF16, tag="xt0a")
        xt0b = xtp.tile([128, 4, UB], BF16, tag="xt0b")
        xts = [None] + [xtp.tile([128, NC8, UB], BF16, tag="xt",
                                 name=f"xt{k}") for k in range(1, NB)]

        def x_rhs(k, c8):
            if k == 0:
                t = xt0a if c8 < 4 else xt0b
                return t[:, c8 % 4, :]
            return xts[k][:, c8, :]

        # PE-gating transfers ride the sync queue in exact need-order (one
        # queue = deterministic priority); wo chunks go on the Act queue
        # since their deadlines are loose
        nc.sync.dma_start(wva[:], wv_d[:, 0:2, :])
        nc.sync.dma_start(xt0a[:], xt_d[0, :, 0:4, :])
        nc.sync.dma_start(wvb[:], wv_d[:, 2:4, :])
        nc.sync.dma_start(xt0b[:], xt_d[0, :, 4:8, :])
        nc.sync.dma_start(wvc[:], wv_d[:, 4:8, :])
        nc.sync.dma_start(xts[1][:], xt_d[1])
        nc.sync.dma_start(xts[2][:], xt_d[2])
        # x block k+3 and wo chunk k: x keeps a two-slot lead over wo
        for k in range(NB - 3):
            nc.sync.dma_start(xts[k + 3][:], xt_d[k + 3])
            nc.sync.dma_start(wo_sb[:, k, :], wo_d[:, k, :])
        for m2 in range(NB - 3, 8):
            nc.sync.dma_start(wo_sb[:, m2, :], wo_d[:, m2, :])

        # vt[h][64*(m%2)+di, 128*(m//2)+s] = v[t=16s+m, 256g+64h+di], bf16
        vt = [vtp.tile([128, D], BF16, tag=f"vt{h}", name=f"vt{h}")
              for h in range(NH)]

        psA = [ps_o.tile([128, 512], F32, tag="po", name=f"psA{h}")
               for h in range(NH)]

        # gpsimd cannot access PSUM, so evacuations go on DVE + Act only
        copy_engines = [nc.vector, nc.scalar]
        ce = [0]

        def ecopy(dst, src):
            eng = copy_engines[ce[0] % 2]
            ce[0] += 1
            if eng is nc.scalar:
                eng.copy(dst, src)
            else:
                eng.tensor_copy(dst, src)

        def vblock(k):
            psv = [ps_v.tile([128, UB], F32, tag="pv", name=f"pv{k}_{hp}")
                   for hp in range(2)]
            for c8 in range(NC8):
                for hp in range(2):
                    nc.tensor.matmul(psv[hp][:], wv_lhs(c8, hp),
                                     x_rhs(k, c8),
                                     start=(c8 == 0), stop=(c8 == NC8 - 1))
            return psv

        def evac(k, psv):
            # block k holds m in {2k, 2k+1}; j = m%2 = local u//128
            for hp in range(2):
                for hh in range(2):
                    for j in range(2):
                        ecopy(vt[2 * hp + hh][64 * j:64 * j + 64,
                                              128 * k:128 * k + 128],
                              psv[hp][64 * hh:64 * hh + 64,
                                      128 * j:128 * j + 128])

        def outA(k):
            for h in range(NH):
                nc.tensor.matmul(psA[h][:], vt[h][:, 128 * k:128 * k + 128],
                                 wo_sb[:, k, 0:512],
                                 start=(k == 0), stop=(k == NB - 1))

        def flushA(h):
            ob = outp.tile([128, 512], BF16, tag="ob", name=f"obA{h}")
            ecopy(ob[:], psA[h][:])
            nc.scalar.dma_start(out_d[h, :, 0:512], ob[:])

        # stream: v-proj chases x DMAs; out-proj chunk k-1 fills PE slack
        psv_prev = vblock(0)
        evac(0, psv_prev)
        for k in range(1, NB):
            psv = vblock(k)
            evac(k, psv)
            outA(k - 1)
        outA(NB - 1)

        # queue all psA evacuations first so the psB bank-reuse waits clear
        # while the first psB groups are still accumulating
        for h in range(NH):
            flushA(h)

        # second pass: out-proj columns [512,1024) + drains
        for h in range(NH):
            psB = ps_o.tile([128, 512], F32, tag="po", name=f"psB{h}")
            for m2 in range(8):
                nc.tensor.matmul(psB[:], vt[h][:, 128 * m2:128 * m2 + 128],
                                 wo_sb[:, m2, 512:1024],
                                 start=(m2 == 0), stop=(m2 == 7))
            ob = outp.tile([128, 512], BF16, tag="ob", name=f"obB{h}")
            ecopy(ob[:], psB[:])
            nc.scalar.dma_start(out_d[h, :, 512:1024], ob[:])


def _get_module():
    global _CACHED
    if _CACHED is None:
        _CACHED = _build_module()
    return _CACHED


def kernel(x, mask, Wq, Wk, Wv, Wo):
    global LAST_RESULTS
    x = np.asarray(x, dtype=np.float32)
    Wv = np.asarray(Wv, dtype=np.float32)
    Wo = np.asarray(Wo, dtype=np.float32)

    b, t, d = x.shape
    assert (b, t, d) == (B, T, D), (b, t, d)

    # x^T with tokens permuted to u = 128m + s (original t = 16s + m),
    # laid out [k, p, c8, u] to match the SBUF tiles exactly
    xts = []
    for bb in range(B):
        xT = x[bb].T                                      # [d, t]
        xTp = xT.reshape(D, 128, 16).transpose(0, 2, 1).reshape(D, T)
        xt = xTp.reshape(NC8, 128, NB, UB).transpose(2, 1, 0, 3)
        xts.append(np.ascontiguousarray(xt).astype(BF))

    # wv[p, c8, col] = Wv[128*c8 + p, col]; per-core slice of 256 cols
    wvp = Wv.reshape(NC8, 128, D).transpose(1, 0, 2)
    # wo[p, m2, n] = Wo.T[128*m2 + p, n]
    woT = np.ascontiguousarray(
        Wo.T.reshape(8, 128, D).transpose(1, 0, 2)).astype(BF)

    in_maps = []
    for c in range(NCORES):
        bb, g = c // 4, c % 4
        in_maps.append({
            "xt": xts[bb],
            "wv": np.ascontiguousarray(
                wvp[:, :, 256 * g:256 * g + 256]).astype(BF),
            "wo": woT,
        })

    nc = _get_module()
    res = run_bass_kernel_spmd(nc, in_maps, list(range(NCORES)))
    LAST_RESULTS = res

    out = np.empty((B, T, D), np.float32)
    for c in range(NCORES):
        bb, g = c // 4, c % 4
        out[bb, 512 * g:512 * g + 512, :] = \
            np.asarray(res.results[c]["out"]).astype(np.float32).reshape(512, D)
    return out


# revision 27
# speedup vs baseline: 1.1060x; 1.0112x over previous
"""nn_MultiHeadAttention_59253368815813 on 8 TRN2 NeuronCores.

The reference module is bug-faithful to its original nn.Module in two ways
that together collapse the computation:

  1. ``o = jnp.einsum('bhtl,bthd->bhtd', A, v)`` indexes ``v`` by the QUERY
     position ``t``, not the key position ``l``. ``l`` therefore only sums
     over the softmax weights, which sum to exactly 1 per row:
     ``o[b,h,t,d] == v[b,t,h,d]``. Q, K, the mask and the softmax never
     influence the output.
  2. ``o.reshape(b, T, d)`` with no transpose scrambles (head, token) so the
     reshaped activation row tj = 128*h + s is the concatenation over
     m=0..15 of v[b, 16*s+m, h, :].

So the exact computation is  out = scramble(x @ Wv) @ Wo.T,  and the
scramble makes output rows depend on one head only.

Sharding: 2 batches x 4 head-groups. Core c = (b=c//4, g=c%4) owns batch b
and heads {4g..4g+3} = Wv columns [256g, 256g+256) and output rows
[512g, 512g+512) of batch b. Each core loads only its batch's x (4.2MB in
bf16) instead of all of x, which is what made the previous version
DMA-bound (23.3MB/core at a shared ~360GB/s).

Per core, all in bf16 (PE runs bf16 at 1 cycle/row like f32r, but DMA
halves; quantization error ~2e-3 << the 2e-2 gate):
  stream x^T (tokens permuted to u = 128m + s, t = 16s + m) in 8 blocks of
  256 tokens; v-proj psum [128,256] per head-pair chases the stream; the
  reshape scramble happens in the psum->SBUF evacuation copies (spread over
  DVE/Pool/Act engines); output-projection columns [0,512) accumulate
  interleaved with the stream (chunk k uses only v tokens of block k);
  columns [512,1024) run as a second pass after the stream, overlapping the
  output DMAs.
"""

import sys
import types

import numpy as np

_TRN_REPO = "/opt/trn_rl_repo"
if _TRN_REPO not in sys.path:
    sys.path.insert(0, _TRN_REPO)


def _install_ntff_shim():
    """antenv.axon_hooks is absent in this container; provide it so
    BASS_TRACE=1 profiling works. No-op if the real module exists."""
    try:
        import antenv  # noqa: F401
    except ImportError:
        return
    if "antenv.axon_hooks" in sys.modules:
        return
    try:
        import antenv.axon_hooks  # noqa: F401
        return
    except ImportError:
        pass
    m = types.ModuleType("antenv.axon_hooks")
    m._hook = None
    m.set_axon_ntff_profile_hook = lambda h: setattr(m, "_hook", h)
    m.get_axon_ntff_profile_hook = lambda: m._hook
    sys.modules["antenv.axon_hooks"] = m
    try:
        from trn_agent_boot.trn_boot import _ntff_profile_via_ctypes

        hook = _ntff_profile_via_ctypes("/opt/axon/libaxon_pjrt.so")
        if hook is not None:
            m.set_axon_ntff_profile_hook(hook)
    except Exception:
        pass


_install_ntff_shim()

import ml_dtypes  # noqa: E402

import concourse.mybir as mybir  # noqa: E402
import concourse.tile as tile  # noqa: E402
from concourse import bacc  # noqa: E402
from concourse.bass_utils import run_bass_kernel_spmd  # noqa: E402

F32 = mybir.dt.float32
BF16 = mybir.dt.bfloat16
BF = ml_dtypes.bfloat16

B = 2
T = 2048
D = 1024
NCORES = 8
NB = 8       # 256-token (u) blocks per batch
UB = 256     # tokens per block
NC8 = 8      # contraction chunks (d = 8*128)
NH = 4       # local heads per core

_CACHED = None
LAST_RESULTS = None


def _build_module():
    nc = bacc.Bacc("TRN2", target_bir_lowering=False, debug=False,
                   num_devices=NCORES)

    xt_d = nc.dram_tensor("xt", [NB, 128, NC8, UB], BF16,
                          kind="ExternalInput").ap()
    wv_d = nc.dram_tensor("wv", [128, NC8, 256], BF16,
                          kind="ExternalInput").ap()
    wo_d = nc.dram_tensor("wo", [128, 8, D], BF16, kind="ExternalInput").ap()
    out_d = nc.dram_tensor("out", [NH, 128, D], BF16,
                           kind="ExternalOutput").ap()

    with tile.TileContext(nc) as tc:
        _emit(nc, tc, xt_d, wv_d, wo_d, out_d)
    nc.compile()
    return nc


def _emit(nc, tc, xt_d, wv_d, wo_d, out_d):
    from contextlib import ExitStack

    ctx = ExitStack()
    with ctx:
        wpool = ctx.enter_context(tc.tile_pool(name="w", bufs=1))
        xtp = ctx.enter_context(tc.tile_pool(name="xt", bufs=NB))
        vtp = ctx.enter_context(tc.tile_pool(name="vt", bufs=1))
        outp = ctx.enter_context(tc.tile_pool(name="outsb", bufs=4))
        ps_v = ctx.enter_context(tc.tile_pool(name="ps_v", bufs=4, space="PSUM"))
        ps_o = ctx.enter_context(tc.tile_pool(name="ps_o", bufs=4, space="PSUM"))

        # PE p-state warmup: the tensor engine clocks up only after ~3us of
        # continuous work, and the first real matmul cannot start until
        # ~10us (runtime preamble + first DMAs). Run throwaway matmuls on a
        # zeroed tile during that window so real matmuls run at full clock.
        warm_sb = wpool.tile([128, 256], BF16, tag="warm")
        nc.vector.memset(warm_sb[:], 0.0)
        warm_ps = ps_v.tile([128, 512], F32, tag="pv", name="warm_ps")
        for _ in range(16):
            nc.tensor.matmul(warm_ps[:, 0:256], warm_sb[:, 0:128],
                             warm_sb[:], start=True, stop=True)

        wva = wpool.tile([128, 2, 256], BF16, tag="wva")
        wvb = wpool.tile([128, 2, 256], BF16, tag="wvb")
        wvc = wpool.tile([128, 4, 256], BF16, tag="wvc")

        def wv_lhs(c8, hp):
            t, i = (wva, c8) if c8 < 2 else (wvb, c8 - 2) if c8 < 4 \
                else (wvc, c8 - 4)
            return t[:, i, 128 * hp:128 * hp + 128]

        wo_sb = wpool.tile([128, 8, D], BF16, tag="wo")
        # block 0 is split in half-tiles so the first v-matmuls start after
        # only half of it (plus wva) has landed
        xt0a = xtp.tile([128, 4, UB], BF16, tag="xt0a")
        xt0b = xtp.tile([128, 4, UB], BF16, tag="xt0b")
        xts = [None] + [xtp.tile([128, NC8, UB], BF16, tag="xt",
                                 name=f"xt{k}") for k in range(1, NB)]

        def x_rhs(k, c8):
            if k == 0:
                t = xt0a if c8 < 4 else xt0b
                return t[:, c8 % 4, :]
            return xts[k][:, c8, :]

        # PE-gating transfers ride the sync queue in exact need-order (one
        # queue = deterministic priority); wo chunks go on the Act queue
        # since their deadlines are loose
        nc.sync.dma_start(wva[:], wv_d[:, 0:2, :])
        nc.sync.dma_start(xt0a[:], xt_d[0, :, 0:4, :])
        nc.sync.dma_start(wvb[:], wv_d[:, 2:4, :])
        nc.sync.dma_start(xt0b[:], xt_d[0, :, 4:8, :])
        nc.sync.dma_start(wvc[:], wv_d[:, 4:8, :])
        nc.sync.dma_start(xts[1][:], xt_d[1])
        nc.sync.dma_start(xts[2][:], xt_d[2])
        # x block k+3 and wo chunk k: x keeps a two-slot lead over wo
        for k in range(NB - 3):
            nc.sync.dma_start(xts[k + 3][:], xt_d[k + 3])
            nc.sync.dma_start(wo_sb[:, k, :], wo_d[:, k, :])
        for m2 in range(NB - 3, 8):
            nc.sync.dma_start(wo_sb[:, m2, :], wo_d[:, m2, :])

        # vt[h][64*(m%2)+di, 128*(m//2)+s] = v[t=16s+m, 256g+64h+di], bf16
        vt = [vtp.tile([128, D], BF16, tag=f"vt{h}", name=f"vt{h}")
              for h in range(NH)]

        psA = [ps_o.tile([128, 512], F32, tag="po", name=f"psA{h}")
               for h in range(NH)]

        # gpsimd cannot access PSUM, so evacuations go on DVE + Act only
        copy_engines = [nc.vector, nc.scalar]
        ce = [0]

        def ecopy(dst, src):
            eng = copy_engines[ce[0] % 2]
            ce[0] += 1
            if eng is nc.scalar:
                eng.copy(dst, src)
            else:
                eng.tensor_copy(dst, src)

        def vblock(k):
            # [128,512] tiles (only half used) so the same pool later hands
            # full banks to the passB psum tiles with no bank-reuse wait
            psv = [ps_v.tile([128, 512], F32, tag="pv", name=f"pv{k}_{hp}")
                   for hp in range(2)]
            for c8 in range(NC8):
                for hp in range(2):
                    nc.tensor.matmul(psv[hp][:, 0:UB], wv_lhs(c8, hp),
                                     x_rhs(k, c8),
                                     start=(c8 == 0), stop=(c8 == NC8 - 1))
            return psv

        def evac(k, psv):
            # block k holds m in {2k, 2k+1}; j = m%2 = local u//128
            for hp in range(2):
                for hh in range(2):
                    for j in range(2):
                        ecopy(vt[2 * hp + hh][64 * j:64 * j + 64,
                                              128 * k:128 * k + 128],
                              psv[hp][64 * hh:64 * hh + 64,
                                      128 * j:128 * j + 128])

        def outA(k):
            for h in range(NH):
                nc.tensor.matmul(psA[h][:], vt[h][:, 128 * k:128 * k + 128],
                                 wo_sb[:, k, 0:512],
                                 start=(k == 0), stop=(k == NB - 1))

        obs = [outp.tile([128, D], BF16, tag="ob", name=f"ob{h}")
               for h in range(NH)]

        def flushA(h):
            ecopy(obs[h][:, 0:512], psA[h][:])

        # stream: v-proj chases x DMAs; out-proj chunk k-1 fills PE slack
        psv_prev = vblock(0)
        evac(0, psv_prev)
        for k in range(1, NB):
            psv = vblock(k)
            evac(k, psv)
            outA(k - 1)
        outA(NB - 1)

        # queue all psA evacuations first so the psB bank-reuse waits clear
        # while the first psB groups are still accumulating
        for h in range(NH):
            flushA(h)

        # second pass: out-proj columns [512,1024); psum banks come from
        # the v pool (free since the stream ended), one whole-head output
        # DMA per head, alternating queues to overlap trigger latency
        for h in range(NH):
            psB = ps_v.tile([128, 512], F32, tag="pv", name=f"psB{h}")
            for m2 in range(8):
                nc.tensor.matmul(psB[:], vt[h][:, 128 * m2:128 * m2 + 128],
                                 wo_sb[:, m2, 512:1024],
                                 start=(m2 == 0), stop=(m2 == 7))
            ecopy(obs[h][:, 512:1024], psB[:])
            eng = nc.scalar if h % 2 == 0 else nc.gpsimd
            eng.dma_start(out_d[h], obs[h][:])


def _get_module():
    global _CACHED
    if _CACHED is None:
        _CACHED = _build_module()
    return _CACHED


def kernel(x, mask, Wq, Wk, Wv, Wo):
    global LAST_RESULTS
    x = np.asarray(x, dtype=np.float32)
    Wv = np.asarray(Wv, dtype=np.float32)
    Wo = np.asarray(Wo, dtype=np.float32)

    b, t, d = x.shape
    assert (b, t, d) == (B, T, D), (b, t, d)

    # x^T with tokens permuted to u = 128m + s (original t = 16s + m),
    # laid out [k, p, c8, u] to match the SBUF tiles exactly
    xts = []
    for bb in range(B):
        xT = x[bb].T                                      # [d, t]
        xTp = xT.reshape(D, 128, 16).transpose(0, 2, 1).reshape(D, T)
        xt = xTp.reshape(NC8, 128, NB, UB).transpose(2, 1, 0, 3)
        xts.append(np.ascontiguousarray(xt).astype(BF))

    # wv[p, c8, col] = Wv[128*c8 + p, col]; per-core slice of 256 cols
    wvp = Wv.reshape(NC8, 128, D).transpose(1, 0, 2)
    # wo[p, m2, n] = Wo.T[128*m2 + p, n]
    woT = np.ascontiguousarray(
        Wo.T.reshape(8, 128, D).transpose(1, 0, 2)).astype(BF)

    in_maps = []
    for c in range(NCORES):
        bb, g = c // 4, c % 4
        in_maps.append({
            "xt": xts[bb],
            "wv": np.ascontiguousarray(
                wvp[:, :, 256 * g:256 * g + 256]).astype(BF),
            "wo": woT,
        })

    nc = _get_module()
    res = run_bass_kernel_spmd(nc, in_maps, list(range(NCORES)))
    LAST_RESULTS = res

    out = np.empty((B, T, D), np.float32)
    for c in range(NCORES):
        bb, g = c // 4, c % 4
        out[bb, 512 * g:512 * g + 512, :] = \
            np.asarray(res.results[c]["out"]).astype(np.float32).reshape(512, D)
    return out
